# revision 24
# baseline (speedup 1.0000x reference)
"""Multi-head attention (B=4, S=2048, D=1024, H=16) on 8 Trainium2 NeuronCores.

Sharding: core c handles batch b=c//2 and head-group g=c%2 (8 heads = 512
features). Per core, transposed dataflow so every matmul contracts over the
SBUF partition dim. All matmul operands are float16 (full PE rate, pipelined
weight loads, ~5e-4 end-to-end rel err), accumulation fp32 in PSUM.

The attention stream is ScalarE(exp)-paced, and engine queues execute in
strict program order, so every projection / output-projection matmul is
broken into small "filler chunks" that are emitted BETWEEN attention
kb-units: the PE queue then always holds dependency-free work to run
inside exp-wait gaps (no head-of-line blocking, HAM clock-gate stays 8/8).

Attention inner unit per (pair t, window qh, key block kb, 512-bank):
one [128,2,512] PSUM tile holds both heads' scores (S matmuls on disjoint
PE row groups run concurrently), one merged exp(S/8) ScalarE op covers
both heads, causal masking via 0/1 bank-masks on diagonal blocks, AV
accumulation with an appended ones column in V producing softmax
denominators in PSUM row 64. Normalization runs per 512-bank as soon as
that bank's last AV lands: denominator row -> repartition DMA
[1,512]->[128,4] -> reciprocal -> flatten DMA -> partition_broadcast ->
multiply, written in place over the dead query columns of QTs[t].
PSUM budget: S(2) + P2 filler(2x1) + ctx_e(2) + ctx_o(2) = 8 banks.
"""

import hashlib
import os
import shutil

import numpy as np

D_MODEL = 1024
N_HEADS = 16
D_K = 64
B = 4
S = 2048
N_CORES = 8
GS = 512            # per-core feature group (8 heads)
NT = GS // 128      # 4 feature tiles (head pairs) per core
NKB = S // 128      # 16 key blocks
W = 1024            # q window width
NW = S // W         # 2 windows

_prog_cache: dict = {}
_last_in_maps = None


def _install_neff_cache():
    import concourse.bass2jax as b2j

    if getattr(b2j, "_ant_neff_cache_installed", False):
        return
    orig = b2j.compile_bir_kernel
    cache_dir = os.environ.get("BASS_NEFF_CACHE", "/tmp/bass_neff_cache")
    os.makedirs(cache_dir, exist_ok=True)

    def cached(bir_json, tmpdir, neff_name="file.neff"):
        data = bir_json if isinstance(bir_json, bytes) else bir_json.encode()
        h = hashlib.sha256(data).hexdigest()[:32]
        cpath = os.path.join(cache_dir, h + ".neff")
        dst = os.path.join(tmpdir, neff_name)
        if os.path.exists(cpath):
            shutil.copyfile(cpath, dst)
            return dst
        out = orig(bir_json, tmpdir, neff_name=neff_name)
        try:
            shutil.copyfile(out, cpath)
        except OSError:
            pass
        return out

    b2j.compile_bir_kernel = cached
    b2j._ant_neff_cache_installed = True


def _rel_start(kb: int, qh: int, mode: str) -> int:
    if mode == "full":
        return 0
    return max(0, kb * 128 - qh * W)


def _build(mode: str):
    import concourse.tile as tile
    from concourse import bacc, mybir

    F16 = mybir.dt.float16
    F32 = mybir.dt.float32
    Exp = mybir.ActivationFunctionType.Exp

    nc = bacc.Bacc("TRN2", target_bir_lowering=False, debug=False,
                   num_devices=N_CORES)
    dp = nc.declare_dram_parameter
    xq = dp("xq", [D_MODEL, S], F16, isOutput=False)
    xk = dp("xk", [D_MODEL, S], F16, isOutput=False)
    xv = dp("xv", [D_MODEL, S], F16, isOutput=False)
    wq = dp("wq", [D_MODEL, GS], F16, isOutput=False)
    wk = dp("wk", [D_MODEL, GS], F16, isOutput=False)
    wv = dp("wv", [D_MODEL, GS], F16, isOutput=False)
    wo = dp("wo", [GS, D_MODEL], F16, isOutput=False)
    bq = dp("bq", [GS], F32, isOutput=False)
    bk = dp("bk", [GS], F32, isOutput=False)
    bv = dp("bv", [1, GS], F16, isOutput=False)
    maskw = dp("maskw", [128, 4, 512], F16, isOutput=False)
    on8 = dp("on8", [128, 8, 1], F16, isOutput=False)
    out = dp("partial", [D_MODEL, S], F16, isOutput=True)

    with tile.TileContext(nc) as tc:
        with tc.tile_pool(name="persist", bufs=1) as persist, \
             tc.tile_pool(name="xpool", bufs=1) as xpool, \
             tc.tile_pool(name="ppool", bufs=1) as ppool, \
             tc.tile_pool(name="psum", bufs=1, space="PSUM") as psum:

            QTs = [persist.tile([128, S], F16, name=f"qts{t}")
                   for t in range(NT)]
            KTs = [persist.tile([128, S], F16, name=f"kts{t}")
                   for t in range(NT)]
            Vhat = [persist.tile([128, 8, 65], F16, name=f"vhat{r}")
                    for r in range(NKB)]
            # ctx for window qh of pair t overwrites QTs[t][:, qh*W:(qh+1)*W]
            # in place: those query columns are dead once window qh's S
            # matmuls have consumed them (WAR tracked by Tile)
            ctx_home = QTs

            bq_sb = persist.tile([128, 4], F32, name="bq_sb")
            bk_sb = persist.tile([128, 4], F32, name="bk_sb")
            bv_row = persist.tile([1, GS], F16, name="bv_row")
            bv_bc = persist.tile([128, GS], F16, name="bv_bc")
            mk_sb = persist.tile([128, 4, 512], F16, name="mk_sb")
            on8_sb = persist.tile([128, 8, 1], F16, name="on8_sb")

            nc.sync.dma_start(out=mk_sb[:], in_=maskw[:])
            nc.sync.dma_start(out=bq_sb[:], in_=bq.rearrange("(m p) -> p m", p=128))
            nc.sync.dma_start(out=bk_sb[:], in_=bk.rearrange("(m p) -> p m", p=128))
            nc.sync.dma_start(out=bv_row[:], in_=bv[:])
            nc.sync.dma_start(out=on8_sb[:], in_=on8[:])
            nc.gpsimd.partition_broadcast(bv_bc[:], bv_row[:])

            # wv first, split per-k so V-proj k=0 starts ASAP
            w_tiles = {}
            wv_t = persist.tile([128, 8, GS], F16, name="wv_t")
            for k in range(8):
                nc.sync.dma_start(out=wv_t[:, k, :],
                                  in_=wv[k * 128:(k + 1) * 128, :])
            w_tiles["wv"] = wv_t

            for z in range(8):
                pz = ppool.tile([128, 2, 512], F16, tag="p", bufs=8,
                                name=f"pzero{z}")
                nc.vector.memset(pz[:], 0.0)

            # HAM warm-up: ~8us of dense dummy matmuls gated only on the
            # small mask DMA, so the PE clock-gate reaches 8/8 while the
            # input DMAs are still streaming in.
            warm_ps = psum.tile([128, 128], F32, tag="P2", bufs=2,
                                name="warm_ps")
            for wi in range(140):
                nc.tensor.matmul(
                    warm_ps[:, (wi % 2) * 64:(wi % 2) * 64 + 64],
                    mk_sb[:, 0, 0:128], mk_sb[:, 0, 0:64],
                    start=True, stop=True)

            ps2_i = [0]

            def p2_ps():
                ps2_i[0] += 1
                return psum.tile([128, 512], F32, tag="P2", bufs=2,
                                 name=f"p2ps{ps2_i[0]}")

            # ---- filler-chunk generators (each chunk emits ~2-4 dep-free
            # PE matmuls or the closing DVE op of a group) ----
            def v_proj_chunks(rg, vsl):
                for ri in range(4):
                    r = rg * 4 + ri
                    box = {}

                    def mm(ks, box=box, ri=ri, r=r):
                        if "pv" not in box:
                            box["pv"] = p2_ps()
                        for k in ks:
                            nc.tensor.matmul(
                                box["pv"][:],
                                vsl[k][:, ri * 128:(ri + 1) * 128],
                                w_tiles["wv"][:, k, :],
                                start=(k == 0), stop=(k == 7))

                    def fin(box=box, r=r):
                        nc.vector.tensor_add(
                            Vhat[r][:, :, 0:64],
                            box["pv"][:].rearrange("p (a b) -> p a b", a=8),
                            bv_bc[:].rearrange("p (a b) -> p a b", a=8))
                        nc.vector.tensor_copy(
                            Vhat[r][:, :, 64:65], on8_sb[:])

                    yield lambda mm=mm: mm(range(0, 4))
                    yield lambda mm=mm: mm(range(4, 8))
                    yield fin

            def v_dma(rg):
                vsl = []
                for k in range(8):
                    s_ = xpool.tile([128, 512], F16, tag="vx", bufs=10,
                                    name=f"vsl{rg}_{k}")
                    nc.sync.dma_start(
                        out=s_[:],
                        in_=xv[k * 128:(k + 1) * 128,
                               rg * 512:(rg + 1) * 512])
                    vsl.append(s_)
                return vsl

            def qk_proj_chunks(t):
                for wname, res, bias_sb_, outs in (
                        ("wq", "xq_res", bq_sb, QTs),
                        ("wk", "xk_res", bk_sb, KTs)):
                    w_t = w_tiles[wname]
                    res = w_tiles[res]
                    for ng in range(2):
                        for hf in range(2):
                            c0 = ng * W + hf * 512
                            box = {}

                            def mm(ks, box=box, c0=c0, w_t=w_t, res=res, t=t):
                                if "pq" not in box:
                                    box["pq"] = p2_ps()
                                for k in ks:
                                    nc.tensor.matmul(
                                        box["pq"][:],
                                        w_t[:, k, t * 128:(t + 1) * 128],
                                        res[k][:, c0:c0 + 512],
                                        start=(k == 0), stop=(k == 7))

                            def fin(box=box, c0=c0, outs=outs, t=t,
                                    bias_sb_=bias_sb_):
                                nc.vector.tensor_scalar_add(
                                    outs[t][:, c0:c0 + 512], box["pq"][:],
                                    bias_sb_[:, t:t + 1])

                            yield lambda mm=mm: mm(range(0, 4))
                            yield lambda mm=mm: mm(range(4, 8))
                            yield fin

            def o_half_chunks(qh):
                for nn in range(2):
                    n = qh * 2 + nn
                    for mo in range(8):
                        box = {}

                        def mm(ts_, box=box, mo=mo, n=n):
                            if "pp" not in box:
                                box["pp"] = p2_ps()
                            for t in ts_:
                                nc.tensor.matmul(
                                    box["pp"][:],
                                    wo_t[:, t, mo * 128:(mo + 1) * 128],
                                    ctx_home[t][:, n * 512:(n + 1) * 512],
                                    start=(t == 0), stop=(t == NT - 1))

                        def fin(box=box, mo=mo, n=n, qh=qh):
                            ot = xpool.tile([128, 512], F16, tag="os",
                                            bufs=4, name=f"ot{qh}_{mo}_{n}")
                            nc.vector.tensor_copy(ot[:], box["pp"][:])
                            nc.sync.dma_start(
                                out=out[mo * 128:(mo + 1) * 128,
                                        n * 512:(n + 1) * 512],
                                in_=ot[:])

                        yield lambda mm=mm: mm(range(NT))
                        yield fin

            filler = []

            def emit_filler(n):
                for _ in range(n):
                    if filler:
                        filler.pop(0)()

            # ---- per-bank softmax normalization for heads (2t, 2t+1) ----
            def normalize(t, qh, hi, ctx_ps, bk_):
                a, b = bk_ * 512, (bk_ + 1) * 512
                po = hi * 64
                sfx = f"{t}_{qh}_{hi}_{bk_}"
                d1 = ppool.tile([1, 512], F32, tag="d1", bufs=2,
                                name=f"d1_{sfx}")
                nc.vector.tensor_copy(d1[:], ctx_ps[64:65, a:b])
                cr = ppool.tile([64, 512], F32, tag="cr", bufs=2,
                                name=f"cr{sfx}")
                nc.vector.tensor_copy(cr[:], ctx_ps[0:64, a:b])
                # ctx psum bank free from here; chain runs off SBUF
                d2 = ppool.tile([128, 4], F32, tag="d2", bufs=2,
                                name=f"d2_{sfx}")
                nc.sync.dma_start(out=d2[:], in_=d1[:])
                d3 = ppool.tile([128, 4], F32, tag="d3", bufs=2,
                                name=f"d3_{sfx}")
                nc.vector.reciprocal(d3[:], d2[:])
                d4 = ppool.tile([1, 512], F32, tag="d4", bufs=2,
                                name=f"d4_{sfx}")
                nc.sync.dma_start(out=d4[:], in_=d3[:])
                bc = ppool.tile([64, 512], F32, tag="bc", bufs=2,
                                name=f"bc{sfx}")
                nc.gpsimd.partition_broadcast(bc[:], d4[:])
                nc.vector.tensor_mul(
                    ctx_home[t][po:po + 64, qh * W + a:qh * W + b],
                    cr[:], bc[:])

            # ---- attention for heads (2t, 2t+1), query window qh ----
            def attention(t, qh, fill=2):
                kbs = [kb for kb in range(NKB)
                       if _rel_start(kb, qh, mode) < W]
                bank_kbs = [[kb for kb in kbs
                             if (_rel_start(kb, qh, mode) // 512) <= bk_]
                            for bk_ in range(W // 512)]
                ctx_e = psum.tile([65, W], F32, tag="ctx_e", bufs=1,
                                  name=f"ctxe{t}_{qh}")
                ctx_o = psum.tile([65, W], F32, tag="ctx_o", bufs=1,
                                  name=f"ctxo{t}_{qh}")
                for kb in kbs:
                    rs = _rel_start(kb, qh, mode)
                    fa = (rs // 512) * 512
                    diag = mode == "tril" and \
                        qh * W <= kb * 128 < (qh + 1) * W
                    mbank = rs // 512
                    pmap = {}
                    for bk_ in range(fa // 512, W // 512):
                        a, b = bk_ * 512, (bk_ + 1) * 512
                        al = max(a, rs) - a
                        # one PSUM tile holds both heads' scores for this
                        # 512-bank; e/o S matmuls run concurrently on
                        # disjoint PE row groups, then a single merged exp
                        # covers both. Only the causal region is computed /
                        # exp'd — sub-rs columns of p are stale-but-finite
                        # and the diagonal-bank mask multiply zeroes them
                        # before AV.
                        s2 = psum.tile([128, 2, 512], F32, tag="S", bufs=1,
                                       name=f"s{t}_{qh}_{kb}_{bk_}")
                        for hi, po in ((0, 0), (1, 64)):
                            nc.tensor.matmul(
                                s2[:, hi, al:512],
                                KTs[t][po:po + 64,
                                       kb * 128:(kb + 1) * 128],
                                QTs[t][po:po + 64,
                                       qh * W + a + al:qh * W + b],
                                start=True, stop=True)
                        p2 = ppool.tile([128, 2, 512], F16, tag="p",
                                        bufs=8, name=f"p{t}_{qh}_{kb}_{bk_}")
                        nc.scalar.activation(p2[:, :, al:512],
                                             s2[:, :, al:512],
                                             Exp, scale=1.0 / 8.0)
                        if diag and bk_ == mbank:
                            for hi in range(2):
                                nc.vector.tensor_mul(
                                    p2[:, hi, :], p2[:, hi, :],
                                    mk_sb[:, kb % 4, :])
                        pmap[bk_] = p2
                        emit_filler(fill)
                    av_banks = sorted(
                        pmap, key=lambda x: (diag and x == mbank, x))
                    for hi, ctx_ps in ((0, ctx_e), (1, ctx_o)):
                        for bk_ in av_banks:
                            a, b = bk_ * 512, (bk_ + 1) * 512
                            nc.tensor.matmul(
                                ctx_ps[:, a:b],
                                Vhat[kb][:, 2 * t + hi, :],
                                pmap[bk_][:, hi, :],
                                start=(kb == bank_kbs[bk_][0]),
                                stop=(kb == bank_kbs[bk_][-1]))
                    for bk_ in range(W // 512):
                        if kb == bank_kbs[bk_][-1]:
                            for hi, ctx_ps in ((0, ctx_e), (1, ctx_o)):
                                normalize(t, qh, hi, ctx_ps, bk_)
                    emit_filler(1)

            # ---- schedule ----
            # window-0 attention needs only key blocks 0-7: run V-proj for
            # them plus pair-0 Q/K projection up front, then stream window-0
            # attention with the remaining projections as PE filler.
            vsl01 = [v_dma(0), v_dma(1)]
            for rg in (0, 1):
                for ch in v_proj_chunks(rg, vsl01[rg]):
                    ch()

            for name, dram in (("wq", wq), ("wk", wk)):
                t_ = persist.tile([128, 8, GS], F16, name=f"{name}_t")
                nc.sync.dma_start(out=t_[:],
                                  in_=dram.rearrange("(k p) m -> p k m", p=128))
                w_tiles[name] = t_
            for nm, dram in (("xq_res", xq), ("xk_res", xk)):
                res = []
                for k in range(8):
                    st = persist.tile([128, S], F16, name=f"{nm}{k}")
                    nc.sync.dma_start(out=st[:],
                                      in_=dram[k * 128:(k + 1) * 128, :])
                    res.append(st)
                w_tiles[nm] = res
            wo_t = persist.tile([128, NT, D_MODEL], F16, name="wo_t")
            nc.sync.dma_start(out=wo_t[:],
                              in_=wo.rearrange("(t p) m -> p t m", p=128))

            for ch in qk_proj_chunks(0):
                ch()

            # qh0 filler: projections for pairs 1-3 and V for blocks 8-15
            vsl23 = [v_dma(2), v_dma(3)]
            filler.extend(qk_proj_chunks(1))
            filler.extend(v_proj_chunks(2, vsl23[0]))
            attention(0, 0, fill=3)
            filler.extend(qk_proj_chunks(2))
            filler.extend(v_proj_chunks(3, vsl23[1]))
            attention(1, 0, fill=3)
            filler.extend(qk_proj_chunks(3))
            attention(2, 0, fill=3)
            attention(3, 0, fill=3)

            # qh1 filler: window-0 output projection (its first chunks
            # depend on the last window-0 normalize, so qh0 leftovers flow
            # into attention(0,1) first, then o-proj chunks feed the rest)
            attention(0, 1, fill=2)
            emit_filler(len(filler))
            filler.extend(o_half_chunks(0))
            attention(1, 1, fill=1)
            attention(2, 1, fill=1)
            attention(3, 1, fill=1)
            emit_filler(len(filler))
            for ch in o_half_chunks(1):
                ch()

    nc.compile()
    return nc


def _get_program(mode: str):
    if mode not in _prog_cache:
        _install_neff_cache()
        _prog_cache[mode] = _build(mode)
    return _prog_cache[mode]


def _make_maskw() -> np.ndarray:
    m = np.zeros((128, 4, 512), np.float16)
    col = np.arange(512)
    for j in range(4):
        o = 128 * j
        for p in range(128):
            m[p, j] = (col >= o + p).astype(np.float16)
    return m


def _numpy_fallback(query, key, value, w_q, b_q, w_k, b_k, w_v, b_v,
                    w_o, b_o, mask):
    def split_heads(x):
        b, s, _ = x.shape
        return x.reshape(b, s, N_HEADS, D_K).transpose(0, 2, 1, 3)

    Q = split_heads(query @ w_q.T + b_q)
    K = split_heads(key @ w_k.T + b_k)
    V = split_heads(value @ w_v.T + b_v)
    out = np.empty((B, N_HEADS, S, D_K), np.float32)
    m2 = np.asarray(mask).reshape(mask.shape[-2], mask.shape[-1])
    for b in range(B):
        for h in range(N_HEADS):
            s = (Q[b, h] @ K[b, h].T) / np.sqrt(np.float32(D_K))
            s = np.where(m2, s, np.finfo(np.float32).min)
            s = s - s.max(axis=-1, keepdims=True)
            e = np.exp(s)
            out[b, h] = (e / e.sum(axis=-1, keepdims=True)) @ V[b, h]
    ctx = out.transpose(0, 2, 1, 3).reshape(B, S, D_MODEL)
    return (ctx @ w_o.T + b_o).astype(np.float32)


def kernel(query, key, value, w_q, b_q, w_k, b_k, w_v, b_v, w_o, b_o, mask):
    query = np.asarray(query, np.float32)
    key = np.asarray(key, np.float32)
    value = np.asarray(value, np.float32)
    w_q, w_k = np.asarray(w_q, np.float32), np.asarray(w_k, np.float32)
    w_v, w_o = np.asarray(w_v, np.float32), np.asarray(w_o, np.float32)
    b_q, b_k = np.asarray(b_q, np.float32), np.asarray(b_k, np.float32)
    b_v, b_o = np.asarray(b_v, np.float32), np.asarray(b_o, np.float32)

    m2 = np.asarray(mask).reshape(mask.shape[-2], mask.shape[-1]).astype(bool)
    if m2.all():
        mode = "full"
    elif np.array_equal(m2, np.tril(np.ones((S, S), bool))):
        mode = "tril"
    else:
        return _numpy_fallback(query, key, value, w_q, b_q, w_k, b_k,
                               w_v, b_v, w_o, b_o, mask)

    from concourse.bass_utils import run_bass_kernel_spmd

    nc = _get_program(mode)

    maskw = _make_maskw()
    on8 = np.ones((128, 8, 1), np.float16)
    f16 = np.float16
    in_maps = []
    for c in range(N_CORES):
        b, g = c // 2, c % 2
        sl = slice(g * GS, (g + 1) * GS)
        in_maps.append({
            "xq": np.ascontiguousarray(query[b].T).astype(f16),
            "xk": np.ascontiguousarray(key[b].T).astype(f16),
            "xv": np.ascontiguousarray(value[b].T).astype(f16),
            "wq": np.ascontiguousarray(w_q[sl, :].T).astype(f16),
            "wk": np.ascontiguousarray(w_k[sl, :].T).astype(f16),
            "wv": np.ascontiguousarray(w_v[sl, :].T).astype(f16),
            "wo": np.ascontiguousarray(w_o[:, sl].T).astype(f16),
            "bq": np.ascontiguousarray(b_q[sl]),
            "bk": np.ascontiguousarray(b_k[sl]),
            "bv": np.ascontiguousarray(b_v[sl][None, :]).astype(f16),
            "maskw": maskw,
            "on8": on8,
        })

    global _last_in_maps
    _last_in_maps = in_maps
    res = run_bass_kernel_spmd(nc, in_maps, list(range(N_CORES)), trace=False)

    out = np.empty((B, S, D_MODEL), np.float32)
    for b in range(B):
        p0 = res.results[2 * b]["partial"].astype(np.float32)
        p1 = res.results[2 * b + 1]["partial"].astype(np.float32)
        out[b] = (p0 + p1).T + b_o
    return out


# revision 26
# speedup vs baseline: 1.0081x; 1.0081x over previous
"""Multi-head attention (B=4, S=2048, D=1024, H=16) on 8 Trainium2 NeuronCores.

Sharding: core c handles batch b=c//2 and head-group g=c%2 (8 heads = 512
features). Per core, transposed dataflow so every matmul contracts over the
SBUF partition dim. All matmul operands are float16 (full PE rate, pipelined
weight loads, ~5e-4 end-to-end rel err), accumulation fp32 in PSUM.

The attention stream is ScalarE(exp)-paced, and engine queues execute in
strict program order, so every projection / output-projection matmul is
broken into small "filler chunks" that are emitted BETWEEN attention
kb-units: the PE queue then always holds dependency-free work to run
inside exp-wait gaps (no head-of-line blocking, HAM clock-gate stays 8/8).

Attention inner unit per (pair t, window qh, key block kb, 512-bank):
one [128,2,512] PSUM tile holds both heads' scores (S matmuls on disjoint
PE row groups run concurrently), one merged exp(S/8) ScalarE op covers
both heads, causal masking via 0/1 bank-masks on diagonal blocks, AV
accumulation with an appended ones column in V producing softmax
denominators in PSUM row 64. Normalization runs per 512-bank as soon as
that bank's last AV lands: denominator row -> repartition DMA
[1,512]->[128,4] -> reciprocal -> flatten DMA -> partition_broadcast ->
multiply, written in place over the dead query columns of QTs[t].
PSUM budget: S(2) + P2 filler(2x1) + ctx_e(2) + ctx_o(2) = 8 banks.
"""

import hashlib
import os
import shutil

import numpy as np

D_MODEL = 1024
N_HEADS = 16
D_K = 64
B = 4
S = 2048
N_CORES = 8
GS = 512            # per-core feature group (8 heads)
NT = GS // 128      # 4 feature tiles (head pairs) per core
NKB = S // 128      # 16 key blocks
W = 1024            # q window width
NW = S // W         # 2 windows

_prog_cache: dict = {}
_last_in_maps = None


def _install_neff_cache():
    import concourse.bass2jax as b2j

    if getattr(b2j, "_ant_neff_cache_installed", False):
        return
    orig = b2j.compile_bir_kernel
    cache_dir = os.environ.get("BASS_NEFF_CACHE", "/tmp/bass_neff_cache")
    os.makedirs(cache_dir, exist_ok=True)

    def cached(bir_json, tmpdir, neff_name="file.neff"):
        data = bir_json if isinstance(bir_json, bytes) else bir_json.encode()
        h = hashlib.sha256(data).hexdigest()[:32]
        cpath = os.path.join(cache_dir, h + ".neff")
        dst = os.path.join(tmpdir, neff_name)
        if os.path.exists(cpath):
            shutil.copyfile(cpath, dst)
            return dst
        out = orig(bir_json, tmpdir, neff_name=neff_name)
        try:
            shutil.copyfile(out, cpath)
        except OSError:
            pass
        return out

    b2j.compile_bir_kernel = cached
    b2j._ant_neff_cache_installed = True


def _rel_start(kb: int, qh: int, mode: str) -> int:
    if mode == "full":
        return 0
    return max(0, kb * 128 - qh * W)


def _build(mode: str):
    import concourse.tile as tile
    from concourse import bacc, mybir

    F16 = mybir.dt.float16
    F32 = mybir.dt.float32
    Exp = mybir.ActivationFunctionType.Exp

    nc = bacc.Bacc("TRN2", target_bir_lowering=False, debug=False,
                   num_devices=N_CORES)
    dp = nc.declare_dram_parameter
    xq = dp("xq", [D_MODEL, S], F16, isOutput=False)
    xk = dp("xk", [D_MODEL, S], F16, isOutput=False)
    xv = dp("xv", [D_MODEL, S], F16, isOutput=False)
    wq = dp("wq", [D_MODEL, GS], F16, isOutput=False)
    wk = dp("wk", [D_MODEL, GS], F16, isOutput=False)
    wv = dp("wv", [D_MODEL, GS], F16, isOutput=False)
    wo = dp("wo", [GS, D_MODEL], F16, isOutput=False)
    bq = dp("bq", [GS], F32, isOutput=False)
    bk = dp("bk", [GS], F32, isOutput=False)
    bv = dp("bv", [1, GS], F16, isOutput=False)
    maskw = dp("maskw", [128, 4, 512], F16, isOutput=False)
    on8 = dp("on8", [128, 8, 1], F16, isOutput=False)
    out = dp("partial", [D_MODEL, S], F16, isOutput=True)

    with tile.TileContext(nc) as tc:
        with tc.tile_pool(name="persist", bufs=1) as persist, \
             tc.tile_pool(name="xpool", bufs=1) as xpool, \
             tc.tile_pool(name="ppool", bufs=1) as ppool, \
             tc.tile_pool(name="psum", bufs=1, space="PSUM") as psum:

            QTs = [persist.tile([128, S], F16, name=f"qts{t}")
                   for t in range(NT)]
            KTs = [persist.tile([128, S], F16, name=f"kts{t}")
                   for t in range(NT)]
            Vhat = [persist.tile([128, 8, 65], F16, name=f"vhat{r}")
                    for r in range(NKB)]
            # ctx for window qh of pair t overwrites QTs[t][:, qh*W:(qh+1)*W]
            # in place: those query columns are dead once window qh's S
            # matmuls have consumed them (WAR tracked by Tile)
            ctx_home = QTs

            bq_sb = persist.tile([128, 4], F32, name="bq_sb")
            bk_sb = persist.tile([128, 4], F32, name="bk_sb")
            bv_row = persist.tile([1, GS], F16, name="bv_row")
            bv_bc = persist.tile([128, GS], F16, name="bv_bc")
            mk_sb = persist.tile([128, 4, 512], F16, name="mk_sb")
            on8_sb = persist.tile([128, 8, 1], F16, name="on8_sb")

            nc.sync.dma_start(out=mk_sb[:], in_=maskw[:])
            nc.sync.dma_start(out=bq_sb[:], in_=bq.rearrange("(m p) -> p m", p=128))
            nc.sync.dma_start(out=bk_sb[:], in_=bk.rearrange("(m p) -> p m", p=128))
            nc.sync.dma_start(out=bv_row[:], in_=bv[:])
            nc.sync.dma_start(out=on8_sb[:], in_=on8[:])
            nc.gpsimd.partition_broadcast(bv_bc[:], bv_row[:])

            # wv first, split per-k so V-proj k=0 starts ASAP
            w_tiles = {}
            wv_t = persist.tile([128, 8, GS], F16, name="wv_t")
            for k in range(8):
                nc.sync.dma_start(out=wv_t[:, k, :],
                                  in_=wv[k * 128:(k + 1) * 128, :])
            w_tiles["wv"] = wv_t

            for z in range(8):
                pz = ppool.tile([128, 2, 512], F16, tag="p", bufs=8,
                                name=f"pzero{z}")
                nc.vector.memset(pz[:], 0.0)

            # HAM warm-up: ~8us of dense dummy matmuls gated only on the
            # small mask DMA, so the PE clock-gate reaches 8/8 while the
            # input DMAs are still streaming in.
            warm_ps = psum.tile([128, 128], F32, tag="P2", bufs=2,
                                name="warm_ps")
            for wi in range(140):
                nc.tensor.matmul(
                    warm_ps[:, (wi % 2) * 64:(wi % 2) * 64 + 64],
                    mk_sb[:, 0, 0:128], mk_sb[:, 0, 0:64],
                    start=True, stop=True)

            ps2_i = [0]

            def p2_ps():
                ps2_i[0] += 1
                return psum.tile([128, 512], F32, tag="P2", bufs=2,
                                 name=f"p2ps{ps2_i[0]}")

            # ---- filler-chunk generators. Each chunk is ATOMIC (psum
            # claim + all 8 matmuls + closing DVE op, ~1.7us of dep-free PE
            # work): nothing is held across attention units, so the P2
            # rotation never head-of-line blocks the engine queue. ----
            def v_proj_chunks(rg, vsl):
                for ri in range(4):
                    def chunk(ri=ri, r=rg * 4 + ri):
                        pv = p2_ps()
                        for k in range(8):
                            nc.tensor.matmul(
                                pv[:],
                                vsl[k][:, ri * 128:(ri + 1) * 128],
                                w_tiles["wv"][:, k, :],
                                start=(k == 0), stop=(k == 7))
                        nc.vector.tensor_add(
                            Vhat[r][:, :, 0:64],
                            pv[:].rearrange("p (a b) -> p a b", a=8),
                            bv_bc[:].rearrange("p (a b) -> p a b", a=8))
                        nc.vector.tensor_copy(
                            Vhat[r][:, :, 64:65], on8_sb[:])
                    yield chunk

            def v_dma(rg):
                vsl = []
                for k in range(8):
                    s_ = xpool.tile([128, 512], F16, tag="vx", bufs=10,
                                    name=f"vsl{rg}_{k}")
                    nc.sync.dma_start(
                        out=s_[:],
                        in_=xv[k * 128:(k + 1) * 128,
                               rg * 512:(rg + 1) * 512])
                    vsl.append(s_)
                return vsl

            def qk_proj_chunks(t):
                for wname, res, bias_sb_, outs in (
                        ("wq", "xq_res", bq_sb, QTs),
                        ("wk", "xk_res", bk_sb, KTs)):
                    for ng in range(2):
                        for hf in range(2):
                            def chunk(w_t=w_tiles[wname],
                                      res=w_tiles[res],
                                      c0=ng * W + hf * 512,
                                      outs=outs, t=t, bias_sb_=bias_sb_):
                                pq = p2_ps()
                                for k in range(8):
                                    nc.tensor.matmul(
                                        pq[:],
                                        w_t[:, k, t * 128:(t + 1) * 128],
                                        res[k][:, c0:c0 + 512],
                                        start=(k == 0), stop=(k == 7))
                                nc.vector.tensor_scalar_add(
                                    outs[t][:, c0:c0 + 512], pq[:],
                                    bias_sb_[:, t:t + 1])
                            yield chunk

            def o_half_chunks(qh):
                for nn in range(2):
                    n = qh * 2 + nn
                    for mo in range(8):
                        def chunk(mo=mo, n=n, qh=qh):
                            pp = p2_ps()
                            for t in range(NT):
                                nc.tensor.matmul(
                                    pp[:],
                                    wo_t[:, t, mo * 128:(mo + 1) * 128],
                                    ctx_home[t][:, n * 512:(n + 1) * 512],
                                    start=(t == 0), stop=(t == NT - 1))
                            ot = xpool.tile([128, 512], F16, tag="os",
                                            bufs=4, name=f"ot{qh}_{mo}_{n}")
                            nc.vector.tensor_copy(ot[:], pp[:])
                            nc.sync.dma_start(
                                out=out[mo * 128:(mo + 1) * 128,
                                        n * 512:(n + 1) * 512],
                                in_=ot[:])
                        yield chunk

            filler = []

            def emit_filler(n):
                for _ in range(n):
                    if filler:
                        filler.pop(0)()

            # ---- per-bank softmax normalization for heads (2t, 2t+1) ----
            def normalize(t, qh, hi, ctx_ps, bk_):
                a, b = bk_ * 512, (bk_ + 1) * 512
                po = hi * 64
                sfx = f"{t}_{qh}_{hi}_{bk_}"
                d1 = ppool.tile([1, 512], F32, tag="d1", bufs=2,
                                name=f"d1_{sfx}")
                nc.vector.tensor_copy(d1[:], ctx_ps[64:65, a:b])
                cr = ppool.tile([64, 512], F32, tag="cr", bufs=2,
                                name=f"cr{sfx}")
                nc.vector.tensor_copy(cr[:], ctx_ps[0:64, a:b])
                # ctx psum bank free from here; chain runs off SBUF
                d2 = ppool.tile([128, 4], F32, tag="d2", bufs=2,
                                name=f"d2_{sfx}")
                nc.sync.dma_start(out=d2[:], in_=d1[:])
                d3 = ppool.tile([128, 4], F32, tag="d3", bufs=2,
                                name=f"d3_{sfx}")
                nc.vector.reciprocal(d3[:], d2[:])
                d4 = ppool.tile([1, 512], F32, tag="d4", bufs=2,
                                name=f"d4_{sfx}")
                nc.sync.dma_start(out=d4[:], in_=d3[:])
                bc = ppool.tile([64, 512], F32, tag="bc", bufs=2,
                                name=f"bc{sfx}")
                nc.gpsimd.partition_broadcast(bc[:], d4[:])
                nc.vector.tensor_mul(
                    ctx_home[t][po:po + 64, qh * W + a:qh * W + b],
                    cr[:], bc[:])

            # ---- attention for heads (2t, 2t+1), query window qh ----
            def attention(t, qh, fill=2):
                kbs = [kb for kb in range(NKB)
                       if _rel_start(kb, qh, mode) < W]
                bank_kbs = [[kb for kb in kbs
                             if (_rel_start(kb, qh, mode) // 512) <= bk_]
                            for bk_ in range(W // 512)]
                ctx_e = psum.tile([65, W], F32, tag="ctx_e", bufs=1,
                                  name=f"ctxe{t}_{qh}")
                ctx_o = psum.tile([65, W], F32, tag="ctx_o", bufs=1,
                                  name=f"ctxo{t}_{qh}")
                for kb in kbs:
                    rs = _rel_start(kb, qh, mode)
                    fa = (rs // 512) * 512
                    diag = mode == "tril" and \
                        qh * W <= kb * 128 < (qh + 1) * W
                    mbank = rs // 512
                    pmap = {}
                    for bk_ in range(fa // 512, W // 512):
                        a, b = bk_ * 512, (bk_ + 1) * 512
                        al = max(a, rs) - a
                        # one PSUM tile holds both heads' scores for this
                        # 512-bank; e/o S matmuls run concurrently on
                        # disjoint PE row groups, then a single merged exp
                        # covers both. Only the causal region is computed /
                        # exp'd — sub-rs columns of p are stale-but-finite
                        # and the diagonal-bank mask multiply zeroes them
                        # before AV.
                        s2 = psum.tile([128, 2, 512], F32, tag="S", bufs=1,
                                       name=f"s{t}_{qh}_{kb}_{bk_}")
                        for hi, po in ((0, 0), (1, 64)):
                            nc.tensor.matmul(
                                s2[:, hi, al:512],
                                KTs[t][po:po + 64,
                                       kb * 128:(kb + 1) * 128],
                                QTs[t][po:po + 64,
                                       qh * W + a + al:qh * W + b],
                                start=True, stop=True)
                        p2 = ppool.tile([128, 2, 512], F16, tag="p",
                                        bufs=8, name=f"p{t}_{qh}_{kb}_{bk_}")
                        nc.scalar.activation(p2[:, :, al:512],
                                             s2[:, :, al:512],
                                             Exp, scale=1.0 / 8.0)
                        if diag and bk_ == mbank:
                            for hi in range(2):
                                nc.vector.tensor_mul(
                                    p2[:, hi, :], p2[:, hi, :],
                                    mk_sb[:, kb % 4, :])
                        pmap[bk_] = p2
                        emit_filler(fill)
                    av_banks = sorted(
                        pmap, key=lambda x: (diag and x == mbank, x))
                    for hi, ctx_ps in ((0, ctx_e), (1, ctx_o)):
                        for bk_ in av_banks:
                            a, b = bk_ * 512, (bk_ + 1) * 512
                            nc.tensor.matmul(
                                ctx_ps[:, a:b],
                                Vhat[kb][:, 2 * t + hi, :],
                                pmap[bk_][:, hi, :],
                                start=(kb == bank_kbs[bk_][0]),
                                stop=(kb == bank_kbs[bk_][-1]))
                    for bk_ in range(W // 512):
                        if kb == bank_kbs[bk_][-1]:
                            for hi, ctx_ps in ((0, ctx_e), (1, ctx_o)):
                                normalize(t, qh, hi, ctx_ps, bk_)
                    emit_filler(1)

            # ---- schedule ----
            # window-0 attention needs only key blocks 0-7: run V-proj for
            # them plus pair-0 Q/K projection up front. Blocks 8-15 are
            # only needed by window 1, so their V projection fills window-1
            # exp gaps. Q/K projections for pairs 1-3 fill window-0 gaps.
            vsl_all = [v_dma(rg) for rg in range(4)]
            for rg in (0, 1):
                for ch in v_proj_chunks(rg, vsl_all[rg]):
                    ch()

            for name, dram in (("wq", wq), ("wk", wk)):
                t_ = persist.tile([128, 8, GS], F16, name=f"{name}_t")
                nc.sync.dma_start(out=t_[:],
                                  in_=dram.rearrange("(k p) m -> p k m", p=128))
                w_tiles[name] = t_
            for nm, dram in (("xq_res", xq), ("xk_res", xk)):
                res = []
                for k in range(8):
                    st = persist.tile([128, S], F16, name=f"{nm}{k}")
                    nc.sync.dma_start(out=st[:],
                                      in_=dram[k * 128:(k + 1) * 128, :])
                    res.append(st)
                w_tiles[nm] = res
            wo_t = persist.tile([128, NT, D_MODEL], F16, name="wo_t")
            nc.sync.dma_start(out=wo_t[:],
                              in_=wo.rearrange("(t p) m -> p t m", p=128))

            for ch in qk_proj_chunks(0):
                ch()

            filler.extend(qk_proj_chunks(1))
            attention(0, 0, fill=1)
            emit_filler(len(filler))      # qk(1) complete before att(1,0)
            filler.extend(qk_proj_chunks(2))
            attention(1, 0, fill=1)
            emit_filler(len(filler))
            filler.extend(qk_proj_chunks(3))
            attention(2, 0, fill=1)
            emit_filler(len(filler))
            # V-proj for key blocks 8-15 (needed from window 1 on) fills
            # att(3,0) and att(0,1); window-0 o-proj fills the rest of qh1
            filler.extend(v_proj_chunks(2, vsl_all[2]))
            attention(3, 0, fill=1)
            filler.extend(v_proj_chunks(3, vsl_all[3]))
            attention(0, 1, fill=1)
            emit_filler(len(filler))      # Vhat 8-15 complete before use
            filler.extend(o_half_chunks(0))
            attention(1, 1, fill=1)
            attention(2, 1, fill=1)
            attention(3, 1, fill=1)
            emit_filler(len(filler))
            for ch in o_half_chunks(1):
                ch()

    nc.compile()
    return nc


def _get_program(mode: str):
    if mode not in _prog_cache:
        _install_neff_cache()
        _prog_cache[mode] = _build(mode)
    return _prog_cache[mode]


def _make_maskw() -> np.ndarray:
    m = np.zeros((128, 4, 512), np.float16)
    col = np.arange(512)
    for j in range(4):
        o = 128 * j
        for p in range(128):
            m[p, j] = (col >= o + p).astype(np.float16)
    return m


def _numpy_fallback(query, key, value, w_q, b_q, w_k, b_k, w_v, b_v,
                    w_o, b_o, mask):
    def split_heads(x):
        b, s, _ = x.shape
        return x.reshape(b, s, N_HEADS, D_K).transpose(0, 2, 1, 3)

    Q = split_heads(query @ w_q.T + b_q)
    K = split_heads(key @ w_k.T + b_k)
    V = split_heads(value @ w_v.T + b_v)
    out = np.empty((B, N_HEADS, S, D_K), np.float32)
    m2 = np.asarray(mask).reshape(mask.shape[-2], mask.shape[-1])
    for b in range(B):
        for h in range(N_HEADS):
            s = (Q[b, h] @ K[b, h].T) / np.sqrt(np.float32(D_K))
            s = np.where(m2, s, np.finfo(np.float32).min)
            s = s - s.max(axis=-1, keepdims=True)
            e = np.exp(s)
            out[b, h] = (e / e.sum(axis=-1, keepdims=True)) @ V[b, h]
    ctx = out.transpose(0, 2, 1, 3).reshape(B, S, D_MODEL)
    return (ctx @ w_o.T + b_o).astype(np.float32)


def kernel(query, key, value, w_q, b_q, w_k, b_k, w_v, b_v, w_o, b_o, mask):
    query = np.asarray(query, np.float32)
    key = np.asarray(key, np.float32)
    value = np.asarray(value, np.float32)
    w_q, w_k = np.asarray(w_q, np.float32), np.asarray(w_k, np.float32)
    w_v, w_o = np.asarray(w_v, np.float32), np.asarray(w_o, np.float32)
    b_q, b_k = np.asarray(b_q, np.float32), np.asarray(b_k, np.float32)
    b_v, b_o = np.asarray(b_v, np.float32), np.asarray(b_o, np.float32)

    m2 = np.asarray(mask).reshape(mask.shape[-2], mask.shape[-1]).astype(bool)
    if m2.all():
        mode = "full"
    elif np.array_equal(m2, np.tril(np.ones((S, S), bool))):
        mode = "tril"
    else:
        return _numpy_fallback(query, key, value, w_q, b_q, w_k, b_k,
                               w_v, b_v, w_o, b_o, mask)

    from concourse.bass_utils import run_bass_kernel_spmd

    nc = _get_program(mode)

    maskw = _make_maskw()
    on8 = np.ones((128, 8, 1), np.float16)
    f16 = np.float16
    in_maps = []
    for c in range(N_CORES):
        b, g = c // 2, c % 2
        sl = slice(g * GS, (g + 1) * GS)
        in_maps.append({
            "xq": np.ascontiguousarray(query[b].T).astype(f16),
            "xk": np.ascontiguousarray(key[b].T).astype(f16),
            "xv": np.ascontiguousarray(value[b].T).astype(f16),
            "wq": np.ascontiguousarray(w_q[sl, :].T).astype(f16),
            "wk": np.ascontiguousarray(w_k[sl, :].T).astype(f16),
            "wv": np.ascontiguousarray(w_v[sl, :].T).astype(f16),
            "wo": np.ascontiguousarray(w_o[:, sl].T).astype(f16),
            "bq": np.ascontiguousarray(b_q[sl]),
            "bk": np.ascontiguousarray(b_k[sl]),
            "bv": np.ascontiguousarray(b_v[sl][None, :]).astype(f16),
            "maskw": maskw,
            "on8": on8,
        })

    global _last_in_maps
    _last_in_maps = in_maps
    res = run_bass_kernel_spmd(nc, in_maps, list(range(N_CORES)), trace=False)

    out = np.empty((B, S, D_MODEL), np.float32)
    for b in range(B):
        p0 = res.results[2 * b]["partial"].astype(np.float32)
        p1 = res.results[2 * b + 1]["partial"].astype(np.float32)
        out[b] = (p0 + p1).T + b_o
    return out
